# revision 8
# baseline (speedup 1.0000x reference)
"""Delay-and-sum beamformer on 8 TRN2 NeuronCores (bf16 fold-tree pipeline).

Problem: x[16, 100000, 128] f32 -> out[b, t] = mean_s x[b, t + d_s, s],
d_s = round(s * sin(30deg) / 2) in [0, 32] (zero-padded past t = T-1).

Sharding: pure data parallel over batch (2 batches per core).

The rel-err gate is 2e-2; bf16 round-to-nearest costs ~7e-3 end to end, so
the host shard step converts to bf16 (folding the 1/128 mean scale into the
conversion -- a power of two, so exact) and the device pipeline is bf16.
This halves DMA bytes and unlocks the DVE bf16 2x path for tensor_tensor.

Per-core layout ("stripe" scheme): for each batch, partition p owns time
rows [784*p, 784*(p+1)) of a zero-padded T_pad = 100384 signal, rows
contiguous in the SBUF free dim as [row, sensor] (row pitch 128 bf16 =
256 B).  Rows stream through a 5-slot ring of 98-row chunks (8 chunks per
batch) in DESCENDING time order, so a chunk's +32-row halo is the already-
resident next chunk at the adjacent slot.  The ring has one 32-row shadow
after slot 4; chunks landing on slot 4 are loaded "fat" (130 rows) so their
window is self-contained.  8 chunks over 5 slots -> first/last chunks of a
batch never collide, so batch handoff does not stall the load queue.

Delay structure: d groups sensors as {0,1,2} (d=0), {4k-1..4k+2} (d=k,
k=1..31), {127} (d=32).  Measured on HW: InstReduce never gets the DVE
bf16 2x mode, InstTensorTensor does (when every operand's innermost dim
is packed), so the 128-tap sum is a binary fold tree of tensor_add:
  DVE:    L1  pair delay groups k=0..14 with k=15..29   (inner [1,4])
          L1b straggler group k=30 folded to 2 values
          L2  62 -> 31, F16 31(+zero col) -> 16, F8 16 -> 8
  GPSIMD: g2t = sensors {0,1,2} @ tau + sensor 127 @ tau+32 (packed dst;
          strided dst writes are ~5x slower on GPSIMD)
          F4/F2/final: 8 -> 1 and += g2t, lagged ONE chunk so GPSIMD
          never head-of-line blocks on the DVE chain.
Loads run on the SP HWDGE queue; output stores run on the Activation
HWDGE queue so they never block loads.  The first fat load is split so
the DVE starts ~5 us earlier; the last chunk is split to shorten drain.
"""

import numpy as np

B, T, S = 16, 100000, 128
NCORES = 8
BC = B // NCORES          # batches per core
LS = 784                  # stripe rows per partition (128*784 = 100352 >= T)
TP = 128 * LS             # padded output rows per batch
HALO = 32                 # max delay
TPAD = TP + HALO          # padded input rows per batch
LC = 98                   # chunk rows
NCH = LS // LC            # 8 chunks per batch
NSLOT = 5                 # ring slots
OFF = 2                   # slot(c) = (c + OFF) % NSLOT; c=7 -> 4 (shadowed)
RING_ROWS = NSLOT * LC + HALO   # 522 rows * 256 B = 133.6 KiB / partition

_cache = {}


def _build():
    import concourse.bass as bass
    import concourse.tile as tile
    from concourse import bacc, mybir

    bf16 = mybir.dt.bfloat16
    nc = bacc.Bacc("TRN2", target_bir_lowering=False, debug=False, num_devices=1)
    x = nc.dram_tensor("x", [BC * TPAD * S], bf16, kind="ExternalInput")
    y = nc.dram_tensor("y", [BC * TP], bf16, kind="ExternalOutput")

    def dram_ap(base_elem, rows):
        # [128 partitions (stripe-major), rows*S contiguous elems each]
        return bass.AP(x.ap().tensor, base_elem, [[LS * S, 128], [1, rows * S]])

    def sub_ap(t, off, dims):
        # custom AP into a tile: keep its partition dim, replace free dims
        return bass.AP(t.tensor, t.offset + off, [list(t.ap[0])] + dims)

    with tile.TileContext(nc) as tc:
        from contextlib import ExitStack

        with ExitStack() as ctx:
            ctx.enter_context(
                nc.allow_low_precision(
                    reason="bf16 sums; rel-err gate is 2e-2, bf16 costs ~7e-3"
                )
            )
            ring_pool = ctx.enter_context(tc.tile_pool(name="ring", bufs=1))
            f_pool = ctx.enter_context(tc.tile_pool(name="f", bufs=1))
            d_pool = ctx.enter_context(tc.tile_pool(name="d", bufs=2))
            o_pool = ctx.enter_context(tc.tile_pool(name="o", bufs=2))

            ring = ring_pool.tile([128, RING_ROWS * S], bf16)
            p1t = f_pool.tile([128, LC * 62], bf16)
            v16t = f_pool.tile([128, LC * 16], bf16)
            # p2 buffers: col 31 stays zero so F16's pair (15,31) is a no-op
            p2a = f_pool.tile([128, LC * 32], bf16)
            p2b = f_pool.tile([128, LC * 32], bf16)
            for p2x in (p2a, p2b):
                nc.vector.memset(sub_ap(p2x, 31, [[32, LC]]), 0.0)

            pending = []  # deferred GPSIMD fold tails (one-chunk lag)

            def compute(ci, c, slot, out_sb, t0=0, t1=LC):
                # produce out rows [c*LC+t0, c*LC+t1) from ring rows
                # [slot*LC+t0, slot*LC+t1+32)
                n = t1 - t0
                base = slot * LC * S + t0 * S
                if pending:  # previous chunk's GPSIMD fold tail (one-chunk lag)
                    pending.pop(0)()
                p2t = p2a if ci % 2 == 0 else p2b
                v8t = d_pool.tile([128, n * 8], bf16, tag="v8")
                g2t = d_pool.tile([128, n], bf16, tag="g2")
                v4t = d_pool.tile([128, n * 4], bf16, tag="v4")
                v2t = d_pool.tile([128, n * 2], bf16, tag="v2")
                # --- DVE chain ---
                nc.vector.tensor_add(
                    sub_ap(p1t, 0, [[62, n], [4, 15], [1, 4]]),
                    sub_ap(ring, base + 131, [[S, n], [132, 15], [1, 4]]),
                    sub_ap(ring, base + 131 + 132 * 15, [[S, n], [132, 15], [1, 4]]),
                )
                nc.vector.tensor_add(
                    sub_ap(p1t, 60, [[62, n], [1, 2]]),
                    sub_ap(ring, base + 131 + 132 * 30, [[S, n], [1, 2]]),
                    sub_ap(ring, base + 133 + 132 * 30, [[S, n], [1, 2]]),
                )
                nc.vector.tensor_add(
                    sub_ap(p2t, 0, [[32, n], [1, 31]]),
                    sub_ap(p1t, 0, [[62, n], [1, 31]]),
                    sub_ap(p1t, 31, [[62, n], [1, 31]]),
                )
                nc.vector.tensor_add(
                    sub_ap(v16t, 0, [[16, n], [1, 16]]),
                    sub_ap(p2t, 0, [[32, n], [1, 16]]),
                    sub_ap(p2t, 16, [[32, n], [1, 16]]),
                )
                nc.vector.tensor_add(
                    sub_ap(v8t, 0, [[8, n], [1, 8]]),
                    sub_ap(v16t, 0, [[16, n], [1, 8]]),
                    sub_ap(v16t, 8, [[16, n], [1, 8]]),
                )
                # --- GPSIMD stragglers (packed dst) ---
                nc.gpsimd.tensor_add(
                    g2t[:],
                    sub_ap(ring, base, [[S, n]]),
                    sub_ap(ring, base + 1, [[S, n]]),
                )
                nc.gpsimd.tensor_add(g2t[:], g2t[:], sub_ap(ring, base + 2, [[S, n]]))
                nc.gpsimd.tensor_add(
                    g2t[:], g2t[:], sub_ap(ring, base + HALO * S + 127, [[S, n]])
                )

                def tail():
                    nc.gpsimd.tensor_add(
                        sub_ap(v4t, 0, [[4, n], [1, 4]]),
                        sub_ap(v8t, 0, [[8, n], [1, 4]]),
                        sub_ap(v8t, 4, [[8, n], [1, 4]]),
                    )
                    nc.gpsimd.tensor_add(
                        sub_ap(v2t, 0, [[2, n], [1, 2]]),
                        sub_ap(v4t, 0, [[4, n], [1, 2]]),
                        sub_ap(v4t, 2, [[4, n], [1, 2]]),
                    )
                    nc.gpsimd.tensor_add(
                        v2t[:, 0:n], sub_ap(v2t, 0, [[2, n]]), sub_ap(v2t, 1, [[2, n]])
                    )
                    nc.gpsimd.tensor_add(
                        out_sb[:, c * LC + t0 : c * LC + t1], v2t[:, 0:n], g2t[:]
                    )

                pending.append(tail)

            def flush_tail():
                while pending:
                    pending.pop(0)()

            ci = 0
            for b in range(BC):
                xb = b * TPAD * S
                out_sb = o_pool.tile([128, LS], bf16, tag="out_sb")
                for c in range(NCH - 1, -1, -1):
                    slot = (c + OFF) % NSLOT
                    rows = LC + HALO if slot == NSLOT - 1 else LC
                    if b == 0 and c == NCH - 1:
                        # very first load: split fat load + compute so the
                        # DVE starts ~5us earlier
                        mid = 49
                        nc.sync.dma_start(
                            sub_ap(ring, (slot * LC + mid) * S, [[1, (rows - mid) * S]]),
                            dram_ap(xb + (c * LC + mid) * S, rows - mid),
                        )
                        compute(ci, c, slot, out_sb, mid, LC)
                        ci += 1
                        nc.sync.dma_start(
                            sub_ap(ring, slot * LC * S, [[1, mid * S]]),
                            dram_ap(xb + c * LC * S, mid),
                        )
                        compute(ci, c, slot, out_sb, 0, mid)
                        ci += 1
                        continue
                    if b == BC - 1 and c == 0:
                        # last chunk: split load+compute to shorten drain
                        mid = 49
                        nc.sync.dma_start(
                            sub_ap(ring, (slot * LC + mid) * S, [[1, (LC - mid) * S]]),
                            dram_ap(xb + (c * LC + mid) * S, LC - mid),
                        )
                        compute(ci, c, slot, out_sb, mid, LC)
                        ci += 1
                        nc.sync.dma_start(
                            sub_ap(ring, slot * LC * S, [[1, mid * S]]),
                            dram_ap(xb + c * LC * S, mid),
                        )
                        compute(ci, c, slot, out_sb, 0, mid)
                        ci += 1
                        continue
                    nc.sync.dma_start(
                        sub_ap(ring, slot * LC * S, [[1, rows * S]]),
                        dram_ap(xb + c * LC * S, rows),
                    )
                    compute(ci, c, slot, out_sb)
                    ci += 1

                flush_tail()
                # store on the Activation HWDGE queue (never blocks loads)
                nc.scalar.dma_start(
                    bass.AP(y.ap().tensor, b * TP, [[LS, 128], [1, LS]]),
                    out_sb[:],
                )

    nc.compile()
    return nc


def _get_nc():
    if "nc" not in _cache:
        _cache["nc"] = _build()
    return _cache["nc"]


def kernel(microphone_array: np.ndarray) -> np.ndarray:
    import ml_dtypes
    from concourse.bass_utils import run_bass_kernel_spmd

    bf16 = np.dtype(ml_dtypes.bfloat16)
    x = np.asarray(microphone_array, dtype=np.float32)
    assert x.shape == (B, T, S)
    nc = _get_nc()

    scale = np.float32(1.0 / S)  # power of two: exact under bf16 rounding
    in_maps = []
    for c in range(NCORES):
        shard = np.zeros((BC, TPAD, S), dtype=bf16)
        shard[:, :T] = (x[c * BC : (c + 1) * BC] * scale).astype(bf16)
        in_maps.append({"x": shard.reshape(-1)})

    res = _cache["res"] = run_bass_kernel_spmd(
        nc, in_maps, core_ids=list(range(NCORES)), trace=_cache.get("trace", False)
    )

    out = np.empty((B, T), dtype=np.float32)
    for c in range(NCORES):
        out[c * BC : (c + 1) * BC] = (
            res.results[c]["y"].reshape(BC, TP)[:, :T].astype(np.float32)
        )
    return out
